# revision 1
# baseline (speedup 1.0000x reference)
"""FFNN-Transducer joint-lattice kernel for 8 Trainium2 NeuronCores.

Ragged-aware decomposition in BOTH lattice axes, uniform-SPMD via phases:
the unit of device work is one (sample, t-block, u-window) tile of 128
encoder frames x Wp joint positions. Only t-blocks with t0 <
encoder_states_size[b] exist, load-balanced across the 8 cores. The
u-axis is split into (up to) two passes chosen by a cost model over the
actual targets_size values: pass 1 covers u in [0, U_mid) for every
valid block; pass 2 covers u in [U_mid, U1e) only for blocks of samples
with targets_size+1 > U_mid (typically a small minority). Each pass is
instruction-uniform across cores; cores with fewer real blocks process
dummies whose output the host ignores.

Per tile the device computes
    out[t,u,:] = tanh(enc_proj[t,:] + pred_bias[u,:]) @ jw2
The tiny prediction network AND the encoder projection enc @ jw1[:E]
(~0.5% of FLOPs) run on host; the host pre-packs, per 16-t span, a
combined lhsT matrix [16 enc_proj rows ; Wp bias rows] so the device
needs only bulk weight DMAs and no prologue.

Device pipeline per core, per (block, pass) tile:
  PE:   per CH-t chunk one "selection" matmul materializing
        A[j,(t,u)] = enc_proj[t,j] + bias[u,j] in PSUM (CH = largest
        divisor of 16 with CH*Wp <= 512, so narrow passes pack more t
        per PSUM bank); per u one [128x128] x [128x88] joint matmul.
  ACT:  batched tanh PSUM->SBUF fp16 (one op per 3-bank A tile).
  DVE:  PSUM->SBUF staging evacuation (fp32->fp16) into [t, (u,v)].
  DMA:  output streamed per u-group pair so no monolithic store ever
        sits ahead of the drain's trailing stores on the FIFO ring.

TRN2 fp32 matmul runs at 1/4 rate, so all TensorE-facing tensors are
fp16; PSUM accumulation stays fp32. The staged output and its
DMA/transfer are fp16 (quantization ~2e-4 abs on a ~0.5-scale output).
jb2-add and the ragged scatter are host epilogues.
"""

import os
import sys

for _p in ("/opt/trn_rl_repo", "/root/.axon_site/_ro/trn_rl_repo"):
    if os.path.isdir(_p) and _p not in sys.path:
        sys.path.append(_p)

import numpy as np

import concourse.bass as bass
import concourse.tile as tile
from concourse import bacc, mybir
from concourse.bass_utils import run_bass_kernel_spmd

# Problem dims (hardcoded per contract)
B, T, E = 8, 1000, 512
U = 100
U1 = U + 1          # 101 joint positions max
H, D, P = 2, 256, 256
J, V = 128, 88
BLANK = V - 1
N_CORES = 8

# Device tiling
TB = 128            # t-steps per block (= joint-matmul lhsT cols, FWL)
SPAN = 16           # t-steps per combined lhsT tile (16 + Wp <= 128 rows)
UG = 5              # u-steps per M-PSUM tile ([128, 512] = 1 bank)

F32 = mybir.dt.float32
F16 = mybir.dt.float16

_CACHE = {}


def _ch_for(w):
    """Largest divisor of SPAN with ch*w <= 512 (PSUM-bank column limit)."""
    ch = SPAN
    while ch > 1 and ch * w > 512:
        ch //= 2
    return ch


def _fronts_for(w):
    """(t_off, n_t) A-tile steps for u-width w: 3 chunks of CH t each."""
    ft = 3 * _ch_for(w)
    steps, t = [], 0
    while t < TB:
        n = min(ft, TB - t)
        steps.append((t, n))
        t += n
    return steps


def _act_block_ns(w):
    """Cost-model ACT time per (block, pass-of-width-w): elements plus
    per-op overhead (PSUM-access init ~185ns + ~90ns engine ack delay)."""
    return 128 * w * 0.8333 + len(_fronts_for(w)) * 275.0


def _plan(tsz, usz):
    """Blocks, u-split and per-core assignment minimizing ACT time."""
    usz1 = usz + 1
    U1e = int(usz1.max())
    blocks = [(b, t0) for b in range(B) for t0 in range(0, int(tsz[b]), TB)]
    if not blocks:
        return None
    K = (len(blocks) + N_CORES - 1) // N_CORES

    # pick up to 2 u-split points minimizing modeled ACT time; candidates
    # are the distinct sequence lengths plus the PSUM chunk-packing
    # breakpoints (u-width <=32 packs 16-t chunks, <=64 packs 8-t)
    cand = sorted({int(x) for x in usz1} | {32, 64})
    cand = [m for m in cand if 0 < m < U1e]

    def cost_of(ms):
        edges = [0] + list(ms) + [U1e]
        total = 0.0
        for u0, u1 in zip(edges[:-1], edges[1:]):
            n = len([blk for blk in blocks if int(usz1[blk[0]]) > u0])
            kp = (n + N_CORES - 1) // N_CORES
            total += kp * _act_block_ns(u1 - u0)
        return total

    best_ms, best_c = (), cost_of(())
    for i, m1 in enumerate(cand):
        for ms in ([(m1,)] + [(m1, m2) for m2 in cand[i + 1:]]):
            c = cost_of(ms)
            if c < best_c:
                best_ms, best_c = ms, c

    def assign(blks, kk):
        padded = blks + [(-1, 0)] * (kk * N_CORES - len(blks))
        return [padded[c * kk:(c + 1) * kk] for c in range(N_CORES)]

    phases = []
    edges = [0] + list(best_ms) + [U1e]
    for u0, u1 in zip(edges[:-1], edges[1:]):
        blks = [blk for blk in blocks if int(usz1[blk[0]]) > u0]
        kp = (len(blks) + N_CORES - 1) // N_CORES
        phases.append((kp, u0, u1 - u0, assign(blks, kp)))
    return U1e, phases


def _build_program(reps=1, geom=None):
    if geom is None:
        geom = _CACHE["geom"]
    # geom: tuple of (Kp, u0p, Wp) per phase
    nc = bacc.Bacc("TRN2", target_bir_lowering=False, debug=False)

    jw2d = nc.dram_tensor("jw2d", [J, V], F16, kind="ExternalInput").ap()
    combd, seld, outd = [], [], []
    for p, (Kp, _u0, Wp) in enumerate(geom):
        combd.append(nc.dram_tensor(
            f"combd{p}", [SPAN + Wp, Kp * (TB // SPAN) * J], F16,
            kind="ExternalInput").ap())
        seld.append(nc.dram_tensor(
            f"seld{p}", [SPAN + Wp, SPAN * Wp], F16, kind="ExternalInput").ap())
        outd.append(nc.dram_tensor(
            f"out{p}", [Kp * TB, Wp * V], F16, kind="ExternalOutput").ap())

    with tile.TileContext(nc) as tc:
        with (
            tc.tile_pool(name="singles", bufs=1) as singles,
            tc.tile_pool(name="hidp", bufs=3) as hidp,
            tc.tile_pool(name="stgp", bufs=2) as stgp,
            tc.tile_pool(name="psA", bufs=2, space="PSUM") as psA,
            tc.tile_pool(name="psM", bufs=2, space="PSUM") as psM,
        ):
            SPB = TB // SPAN
            comb_sb, sel_sb = [], []
            for p, (Kp, _u0, Wp) in enumerate(geom):
                comb_t = singles.tile([SPAN + Wp, Kp * SPB * J], F16,
                                      tag=f"comb{p}", name=f"comb_t{p}")
                sel_t = singles.tile([SPAN + Wp, SPAN * Wp], F16,
                                     tag=f"sel{p}", name=f"sel_t{p}")
                comb_sb.append(comb_t)
                sel_sb.append(sel_t)
            # first block's lhsT + its sel load first (parallel HWDGE
            # queues) so the pipeline starts after ~2 small DMAs
            nc.sync.dma_start(out=comb_sb[0][:, 0:SPB * J],
                              in_=combd[0][:, 0:SPB * J])
            nc.scalar.dma_start(out=sel_sb[0][:, :], in_=seld[0][:, :])
            jw2_sb = singles.tile([J, V], F16, tag="jw2")
            nc.sync.dma_start(out=jw2_sb[:, :], in_=jw2d[:, :])
            for blk in range(1, geom[0][0]):
                nc.sync.dma_start(
                    out=comb_sb[0][:, blk * SPB * J:(blk + 1) * SPB * J],
                    in_=combd[0][:, blk * SPB * J:(blk + 1) * SPB * J],
                )
            for p in range(1, len(geom)):
                nc.sync.dma_start(out=comb_sb[p][:, :], in_=combd[p][:, :])
                nc.scalar.dma_start(out=sel_sb[p][:, :], in_=seld[p][:, :])

            for rep in range(reps):
                _emit_rep(nc, hidp, stgp, psA, psM, comb_sb, jw2_sb, sel_sb,
                          outd, rep, geom, last_rep=(rep == reps - 1))

    nc.compile()
    return nc


def _emit_rep(nc, hidp, stgp, psA, psM, comb_sb, jw2_sb, sel_sb, outd, rep,
              geom, last_rep=True):
    fronts = [_fronts_for(Wp) for (_K, _u0, Wp) in geom]
    chs = [_ch_for(Wp) for (_K, _u0, Wp) in geom]
    nugs = [(Wp + UG - 1) // UG for (_K, _u0, Wp) in geom]
    # interleave phases by fractional position: narrow (front-poor) entries
    # land between wide ones, whose fronts absorb their neighbours' backs
    ext = [(p, li) for p, (Kp, _u0, _W) in enumerate(geom) for li in range(Kp)]
    ext.sort(key=lambda e: ((e[1] + 0.5) / geom[e[0]][0], e[0]))
    hid_t, stg_t = {}, {}

    A_t = {}

    def front_mm(p, li, fi):
        W = geom[p][2]
        CH = chs[p]
        t_off, n_t = fronts[p][fi]
        if fi == 0:
            hid_t[(p, li)] = hidp.tile([128, W * TB], F16, tag=f"hid{p}",
                                       name=f"hid{rep}_{p}_{li}")
        A = psA.tile([128, 1536], F32, tag="A", name=f"A{rep}_{p}_{li}_{fi}")
        A_t[(p, li, fi)] = A
        nch = n_t // CH
        for c in range(nch):
            tg = t_off + c * CH
            sp = li * (TB // SPAN) + tg // SPAN
            tl = tg % SPAN
            nc.tensor.matmul(
                A[:, c * 512:c * 512 + CH * W],
                comb_sb[p][:, sp * J:(sp + 1) * J],
                sel_sb[p][:, tl * W:(tl + CH) * W],
                start=True,
                stop=True,
            )

    def front_tanh(p, li, fi):
        W = geom[p][2]
        CH = chs[p]
        t_off, n_t = fronts[p][fi]
        hid2 = hid_t[(p, li)]
        A = A_t.pop((p, li, fi))
        nch = n_t // CH
        nc.scalar.activation(
            out=hid2[:, t_off * W:(t_off + n_t) * W].rearrange(
                "p (c x) -> p c x", c=nch),
            in_=A.rearrange("p (c x) -> p c x", c=3)[:, 0:nch, 0:CH * W],
            func=mybir.ActivationFunctionType.Tanh,
        )

    def back(p, li, ug, is_last_block):
        W = geom[p][2]
        NUG = nugs[p]
        hid2 = hid_t[(p, li)]
        if ug == 0:
            stg_t[(p, li)] = stgp.tile([TB, W * V], F16, tag=f"stg{p}",
                                       name=f"stg{rep}_{p}_{li}")
        stg = stg_t[(p, li)]
        u0 = ug * UG
        n_u = min(UG, W - u0)
        M = psM.tile([TB, 512], F32, tag="M", name=f"M{rep}_{p}_{li}_{ug}")
        hid_ut = hid2.rearrange("p (t u) -> p u t", u=W)  # strided view
        for i in range(n_u):
            nc.tensor.matmul(
                M[:, i * V:(i + 1) * V],
                hid_ut[:, u0 + i, :],
                jw2_sb[:, :],
                start=True,
                stop=True,
            )
        if last_rep and is_last_block and ug % 2 == 1:
            # final drain: ScalarE takes alternate evacuations (it is idle
            # after the very last tanh) so the drain runs two parallel chains
            nc.scalar.copy(
                out=stg[:, u0 * V:(u0 + n_u) * V],
                in_=M[:, 0:n_u * V],
            )
        else:
            nc.vector.tensor_copy(
                out=stg[:, u0 * V:(u0 + n_u) * V],
                in_=M[:, 0:n_u * V],
            )
        if ug % 2 == 1 or ug == NUG - 1:
            # stream output per u-group pair: no monolithic DMA ever blocks
            # the FIFO ring ahead of the drain's small trailing stores
            u_lo = (ug // 2) * 2 * UG
            u_hi = u0 + n_u
            nc.sync.dma_start(
                out=outd[p][li * TB:(li + 1) * TB, u_lo * V:u_hi * V],
                in_=stg[:, u_lo * V:u_hi * V],
            )

    # software-pipelined emission with globally-paced lazy backs: a back
    # becomes eligible once its entry's fronts are all emitted, and the
    # eligible FIFO drains proportionally across ALL front slots, so a
    # narrow phase's few fronts never absorb a wide block's backs alone
    total_fronts = sum(len(fronts[p]) for p, _li in ext)
    total_backs = sum(nugs[p] for p, _li in ext)
    pending = []
    emitted = 0
    gfi = 0
    for idx, (p, li) in enumerate(ext):
        nf = len(fronts[p])
        for fi in range(nf):
            front_mm(p, li, fi)
            front_tanh(p, li, fi)
            gfi += 1
            target = gfi * total_backs // total_fronts
            while pending and emitted < target:
                bp, bl, bug = pending.pop(0)
                back(bp, bl, bug, False)
                emitted += 1
        pending.extend((p, li, ug) for ug in range(nugs[p]))
    lp, ll = ext[-1]
    for bp, bl, bug in pending:
        back(bp, bl, bug, (bp, bl) == (lp, ll))


def _host_pred_bias(targets_b, emb, pw1, pb1, pw2, pb2, jw1, jb1):
    """bias[u, j] = (pred @ jw1[E:] + jb1)[u, j] for the U1 joint positions."""
    ext = np.concatenate([np.full(H, BLANK, np.int64), targets_b.astype(np.int64)])
    e = np.concatenate([emb[ext[1:U1 + 1]], emb[ext[0:U1]]], axis=1)  # [101, 512]
    h = np.tanh(e @ pw1 + pb1)
    pred = np.tanh(h @ pw2 + pb2)
    return (pred @ jw1[E:] + jb1).astype(np.float32)  # [101, 128]


def _make_sel(w):
    sel = np.zeros((SPAN + w, SPAN * w), np.float16)
    for tl in range(SPAN):
        sel[tl, tl * w:(tl + 1) * w] = 1.0
        sel[SPAN:SPAN + w, tl * w:(tl + 1) * w] += np.eye(w, dtype=np.float16)
    return sel


def _make_in_maps(encoder_states, targets, emb, pw1, pb1, pw2, pb2, jw1, jb1,
                  jw2, U1e, phases):
    encoder_states = np.asarray(encoder_states, dtype=np.float32)
    jw1 = np.asarray(jw1, dtype=np.float32)
    jw2_np = np.ascontiguousarray(np.asarray(jw2, dtype=np.float32)).astype(np.float16)
    jw1enc = np.ascontiguousarray(jw1[:E])

    # host: encoder projection (fp32 GEMM) + prediction-network bias
    eproj = np.zeros((B, T, J), np.float16)
    for b in range(B):
        eproj[b] = (encoder_states[b] @ jw1enc).astype(np.float16)
    bias_all = np.empty((B, U1e, J), np.float16)
    for b in range(B):
        bias_all[b] = _host_pred_bias(
            np.asarray(targets[b]), np.asarray(emb, np.float32),
            np.asarray(pw1, np.float32), np.asarray(pb1, np.float32),
            np.asarray(pw2, np.float32), np.asarray(pb2, np.float32),
            jw1, np.asarray(jb1, np.float32),
        )[:U1e].astype(np.float16)

    sels = [_make_sel(p[2]) for p in phases]

    in_maps = []
    for c in range(N_CORES):
        m = {"jw2d": jw2_np}
        for p, (Kp, u0p, Wp, core_blocks) in enumerate(phases):
            KC = SPAN + Wp
            combd = np.zeros((KC, Kp * (TB // SPAN) * J), np.float16)
            for k, (b, t0) in enumerate(core_blocks[c]):
                if b < 0:
                    continue  # dummy block: zeros are fine, output ignored
                for si in range(TB // SPAN):
                    sp = k * (TB // SPAN) + si
                    ts = t0 + si * SPAN
                    te = min(ts + SPAN, T)
                    if ts < te:
                        combd[0:te - ts, sp * J:sp * J + J] = eproj[b, ts:te]
                    combd[SPAN:KC, sp * J:(sp + 1) * J] = \
                        bias_all[b, u0p:u0p + Wp]
            m[f"combd{p}"] = combd
            m[f"seld{p}"] = sels[p]
        in_maps.append(m)
    return in_maps


def kernel(encoder_states, encoder_states_size, targets, targets_size,
           emb, pw1, pb1, pw2, pb2, jw1, jb1, jw2, jb2):
    tsz = np.asarray(encoder_states_size).astype(np.int64)
    usz = np.asarray(targets_size).astype(np.int64)
    plan = _plan(tsz, usz)
    if plan is None:  # no valid lattice positions anywhere
        return np.zeros((B, T, U1, V), np.float32)
    U1e, phases = plan
    geom = tuple((Kp, u0p, Wp) for (Kp, u0p, Wp, _cb) in phases)

    if _CACHE.get("geom") != geom:
        _CACHE.clear()
        _CACHE["geom"] = geom
    if "nc" not in _CACHE:
        _CACHE["nc"] = _build_program(reps=1, geom=geom)
    nc = _CACHE["nc"]

    in_maps = _make_in_maps(encoder_states, targets, emb, pw1, pb1, pw2, pb2,
                            jw1, jb1, jw2, U1e, phases)
    _CACHE["in_maps"] = in_maps
    res = run_bass_kernel_spmd(nc, in_maps, core_ids=list(range(N_CORES)))

    jb2f = np.asarray(jb2, np.float32)
    out = np.zeros((B, T, U1, V), np.float32)
    for c in range(N_CORES):
        for p, (Kp, u0p, Wp, core_blocks) in enumerate(phases):
            dev = res.results[c][f"out{p}"]  # [Kp*TB, Wp*V] fp16
            for k, (b, t0) in enumerate(core_blocks[c]):
                if b < 0:
                    continue
                nt = min(TB, int(tsz[b]) - t0)
                u1b = int(usz[b]) + 1
                w = min(Wp, u1b - u0p)
                if w <= 0:
                    continue
                blk = dev[k * TB:k * TB + nt].reshape(nt, Wp, V)[:, :w, :]
                out[b, t0:t0 + nt, u0p:u0p + w, :] = \
                    blk.astype(np.float32) + jb2f
    return out



# revision 20
# speedup vs baseline: 1.4865x; 1.4865x over previous
"""FFNN-Transducer joint-lattice kernel for 8 Trainium2 NeuronCores.

Sorted-span-row decomposition: the unit of work is a 16-frame encoder
span (b, ts). All spans of the batch are sorted by target length
(usz+1) descending and packed into rows of 64 spans (8 cores x 8
spans); each row's joint width W is the max usz+1 within the row, so
u-padding adapts to the data with no fixed phase structure. Spans from
different samples share a row: the combined lhsT carries, per span, 16
encoder-projection rows plus that span's own prediction-bias rows, so
blocks mix samples freely and the t-axis is packed at 16-frame
granularity. The final partial row runs with TB = 16*ceil(left/8)
frames; its joint matmul is emitted v-partitioned (out[v, (t,u)]) so
the drain tail stays ~1us.

Per tile the device computes
    out[t,u,:] = tanh(enc_proj[t,:] + pred_bias[u,:]) @ jw2
with the tiny prediction network and the encoder projection enc@jw1[:E]
(~0.5% of FLOPs) on host.

Device pipeline per core, per row:
  PE:   per CH-frame chunk one "selection" matmul materializing
        A[j,(t,u)] = enc_proj[t,j] + bias[u,j] in PSUM; per u one
        [128x128] x [128x88] joint matmul (t-form), or jw2-stationary
        [128x88]^T @ hid for the small tail row (v-form).
  ACT:  batched tanh PSUM->SBUF fp16, one op per 3-bank A tile. A
        warmup tanh at t=0 preloads the activation table during the
        initial DMAs.
  DVE:  PSUM->SBUF evacuation (fp32->fp16); back-emission is paced by
        modeled ACT/DVE time so neither engine starves at row edges.
  DMA:  outputs streamed per u-group pair on the sync queue; selection
        matrices ride the otherwise-idle gpsimd (SWDGE) queue.

TRN2 fp32 matmul runs at 1/4 rate, so all TensorE-facing tensors are
fp16; PSUM stays fp32. jb2-add and the ragged scatter are host
epilogues.
"""

import os
import sys

for _p in ("/opt/trn_rl_repo", "/root/.axon_site/_ro/trn_rl_repo"):
    if os.path.isdir(_p) and _p not in sys.path:
        sys.path.append(_p)

import numpy as np

import concourse.bass as bass
import concourse.tile as tile
from concourse import bacc, mybir
from concourse.bass_utils import run_bass_kernel_spmd

# Problem dims (hardcoded per contract)
B, T, E = 8, 1000, 512
U = 100
U1 = U + 1
H, D, P = 2, 256, 256
J, V = 128, 88
BLANK = V - 1
N_CORES = 8

SPAN = 16           # t-frames per span (lhsT packing unit)
SPB = 8             # spans per core per full row (TB = 128)
UG = 5              # u-steps per M-PSUM bank in t-form backs

F32 = mybir.dt.float32
F16 = mybir.dt.float16

_CACHE = {}


def _ch_for(w):
    """Largest divisor of SPAN with ch*w <= 512 (PSUM-bank column limit)."""
    ch = SPAN
    while ch > 1 and ch * w > 512:
        ch //= 2
    return ch


def _fronts_for(w, tb):
    """(t_off, n_t) A-tile steps covering tb frames: 3 chunks of CH each."""
    ft = 3 * _ch_for(w)
    steps, t = [], 0
    while t < tb:
        n = min(ft, tb - t)
        steps.append((t, n))
        t += n
    return steps


def _plan(tsz, usz):
    """Sort spans by target length, pack into rows of 64.

    Returns (U1e, rows); rows = [(TB, W, vform, assign)] where assign
    is the per-core list of (sample, t_start) spans (-1 = dummy).
    """
    usz1 = usz + 1
    spans = [(b, ts) for b in range(B) for ts in range(0, int(tsz[b]), SPAN)]
    if not spans:
        return None
    spans.sort(key=lambda s: (-int(usz1[s[0]]), s[0], s[1]))
    per_row = N_CORES * SPB
    rows = []
    i = 0
    while i < len(spans):
        chunk = spans[i:i + per_row]
        spr = (len(chunk) + N_CORES - 1) // N_CORES
        W = int(usz1[chunk[0][0]])
        chunk = chunk + [(-1, 0)] * (N_CORES * spr - len(chunk))
        assign = [chunk[c * spr:(c + 1) * spr] for c in range(N_CORES)]
        TB = spr * SPAN
        vform = TB < V  # short rows evacuate fewer cols v-partitioned
        rows.append((TB, W, vform, assign))
        i += per_row
    return int(usz1.max()), rows


def _sections(rows):
    """Split wide rows at u=64 so A-chunks fill PSUM banks (CH stays >= 8,
    fewer/larger ACT ops). Each section shares its row's comb lhsT; the
    selection matrix picks the section's bias rows.

    Returns [(row_id, TB, Wrow, u0, Ws, vform)].
    """
    secs = []
    for r, (TB, W, vform, _assign) in enumerate(rows):
        if W > 64 and not vform:
            # (W1<=64, W2<=32) halves CH-chunk count: 6+3 ACT ops per
            # 128 frames instead of 11, with no tiny sections
            w2 = min(32, W - 33)
            secs.append((r, TB, W, 0, W - w2, False))
            secs.append((r, TB, W, W - w2, w2, False))
        else:
            secs.append((r, TB, W, 0, W, vform))
    return secs


def _build_program(reps=1, geom=None):
    if geom is None:
        geom = _CACHE["geom"]
    # geom: tuple of (row_id, TB, Wrow, u0, Ws, vform) per section
    nc = bacc.Bacc("TRN2", target_bir_lowering=False, debug=False)

    jw2d = nc.dram_tensor("jw2d", [J, V], F16, kind="ExternalInput").ap()
    row_ids = []
    row_dims = {}
    sel_keys = []
    outd = []
    for p, (r, TBp, Wr, u0, Ws, vf) in enumerate(geom):
        if r not in row_ids:
            row_ids.append(r)
            row_dims[r] = (TBp, Wr)
        key = (Wr, u0, Ws)
        if key not in sel_keys:
            sel_keys.append(key)
        oshape = [V, TBp * Ws] if vf else [TBp, Ws * V]
        outd.append(nc.dram_tensor(
            f"out{p}", oshape, F16, kind="ExternalOutput").ap())
    combd = {r: nc.dram_tensor(
        f"combd{r}", [SPAN + row_dims[r][1], (row_dims[r][0] // SPAN) * J],
        F16, kind="ExternalInput").ap() for r in row_ids}
    seld = {k: nc.dram_tensor(f"selw{k[0]}_{k[1]}_{k[2]}",
                              [SPAN + k[0], SPAN * k[2]], F16,
                              kind="ExternalInput").ap()
            for k in sel_keys}

    hid_max = max(TBp * Ws for (_r, TBp, _Wr, _u0, Ws, _vf) in geom)
    stg_max = max((Ws * V if not vf else TBp * Ws)
                  for (_r, TBp, _Wr, _u0, Ws, vf) in geom)

    with tile.TileContext(nc) as tc:
        with (
            tc.tile_pool(name="singles", bufs=1) as singles,
            tc.tile_pool(name="hidp", bufs=4) as hidp,
            tc.tile_pool(name="stgp", bufs=3) as stgp,
            tc.tile_pool(name="psA", bufs=2, space="PSUM") as psA,
            tc.tile_pool(name="psM", bufs=2, space="PSUM") as psM,
        ):
            # warmup: preload the tanh table set while the first DMAs fly
            warm = singles.tile([128, 2], F16, tag="warm", name="warm")
            warm2 = singles.tile([128, 2], F16, tag="warm2", name="warm2")
            nc.vector.memset(warm[:, :], 0.0)
            nc.scalar.activation(out=warm2[:, :], in_=warm[:, :],
                                 func=mybir.ActivationFunctionType.Tanh)

            comb_sb = {}
            sel_sb = {}
            for r in row_ids:
                TBr, Wr = row_dims[r]
                comb_sb[r] = singles.tile(
                    [SPAN + Wr, (TBr // SPAN) * J], F16, tag=f"comb{r}",
                    name=f"comb_t{r}")
            for k in sel_keys:
                sel_sb[k] = singles.tile([SPAN + k[0], SPAN * k[2]], F16,
                                         tag=f"selw{k}", name=f"sel_t{k}")
            jw2_sb = singles.tile([J, V], F16, tag="jw2", name="jw2_sb")

            # first row's inputs first (parallel queues: comb on sync,
            # sel on the idle gpsimd/SWDGE queue), then everything else;
            # the first span's lhsT slice leads so the pipeline starts
            # after one tiny DMA instead of the full row load
            r0 = row_ids[0]
            nc.sync.dma_start(out=comb_sb[r0][:, 0:J], in_=combd[r0][:, 0:J])
            nc.sync.dma_start(out=comb_sb[r0][:, J:], in_=combd[r0][:, J:])
            nc.gpsimd.dma_start(out=sel_sb[sel_keys[0]][:, :],
                                in_=seld[sel_keys[0]][:, :])
            nc.sync.dma_start(out=jw2_sb[:, :], in_=jw2d[:, :])
            for k in sel_keys[1:]:
                nc.gpsimd.dma_start(out=sel_sb[k][:, :], in_=seld[k][:, :])
            for r in row_ids[1:]:
                nc.sync.dma_start(out=comb_sb[r][:, :], in_=combd[r][:, :])

            for rep in range(reps):
                _emit_rep(nc, hidp, stgp, psA, psM, comb_sb, jw2_sb, sel_sb,
                          outd, rep, geom, hid_max, stg_max,
                          last_rep=(rep == reps - 1))

    nc.compile()
    return nc


def _emit_rep(nc, hidp, stgp, psA, psM, comb_sb, jw2_sb, sel_sb, outd, rep,
              geom, hid_max, stg_max, last_rep=True):
    fronts = [_fronts_for(Ws, TBp) for (_r, TBp, _Wr, _u0, Ws, _vf) in geom]
    chs = [_ch_for(Ws) for (_r, _TB, _Wr, _u0, Ws, _vf) in geom]

    hid_t, stg_t, A_t = {}, {}, {}

    def front_mm(p, fi):
        r, TBp, Wr, u0, Ws, _vf = geom[p]
        CH = chs[p]
        t_off, n_t = fronts[p][fi]
        if fi == 0:
            hid_t[p] = hidp.tile([128, hid_max], F16, tag="hid",
                                 name=f"hid{rep}_{p}")
        A = psA.tile([128, 1536], F32, tag="A", name=f"A{rep}_{p}_{fi}")
        A_t[(p, fi)] = A
        sel = sel_sb[(Wr, u0, Ws)]
        for c in range(n_t // CH):
            tg = t_off + c * CH
            sp, tl = tg // SPAN, tg % SPAN
            nc.tensor.matmul(
                A[:, c * 512:c * 512 + CH * Ws],
                comb_sb[r][:, sp * J:(sp + 1) * J],
                sel[:, tl * Ws:(tl + CH) * Ws],
                start=True,
                stop=True,
            )

    def front_tanh(p, fi):
        _r, _TBp, _Wr, _u0, Ws, _vf = geom[p]
        CH = chs[p]
        t_off, n_t = fronts[p][fi]
        nch = n_t // CH
        A = A_t.pop((p, fi))
        nc.scalar.activation(
            out=hid_t[p][:, t_off * Ws:(t_off + n_t) * Ws].rearrange(
                "p (c x) -> p c x", c=nch),
            in_=A.rearrange("p (c x) -> p c x", c=3)[:, 0:nch, 0:CH * Ws],
            func=mybir.ActivationFunctionType.Tanh,
        )

    def n_backs(p):
        _r, TBp, _Wr, _u0, Ws, vf = geom[p]
        return ((TBp * Ws + 511) // 512) if vf else ((Ws + UG - 1) // UG)

    def back_cost(p, bi):
        _r, TBp, _Wr, _u0, Ws, vf = geom[p]
        if vf:
            n = min(512, TBp * Ws - bi * 512)
            return n * 1.05 + 130
        n_u = min(UG, Ws - bi * UG)
        return n_u * V * 1.05 + 130

    def back(p, bi, use_scalar):
        _r, TBp, _Wr, _u0, Ws, vf = geom[p]
        hid2 = hid_t[p]
        if bi == 0:
            stg_t[p] = stgp.tile([128, stg_max], F16, tag="stg",
                                 name=f"stg{rep}_{p}")
        stg = stg_t[p]
        M = psM.tile([128, 512], F32, tag="M", name=f"M{rep}_{p}_{bi}")
        cp = nc.scalar.copy if use_scalar else nc.vector.tensor_copy
        if vf:
            c0 = bi * 512
            n = min(512, TBp * Ws - c0)
            nc.tensor.matmul(
                M[0:V, 0:n],
                jw2_sb[:, :],
                hid2[:, c0:c0 + n],
                start=True,
                stop=True,
            )
            cp(out=stg[0:V, c0:c0 + n], in_=M[0:V, 0:n])
            if (bi + 1) * 512 >= TBp * Ws:
                nc.sync.dma_start(out=outd[p][:, :],
                                  in_=stg[0:V, 0:TBp * Ws])
            return
        NUGp = n_backs(p)
        ug0 = bi * UG
        n_u = min(UG, Ws - ug0)
        hid_ut = hid2[:, 0:TBp * Ws].rearrange("p (t u) -> p u t", u=Ws)
        for i in range(n_u):
            nc.tensor.matmul(
                M[0:TBp, i * V:(i + 1) * V],
                hid_ut[:, ug0 + i, :],
                jw2_sb[:, :],
                start=True,
                stop=True,
            )
        cp(out=stg[0:TBp, ug0 * V:(ug0 + n_u) * V], in_=M[0:TBp, 0:n_u * V])
        if bi % 2 == 1 or bi == NUGp - 1:
            u_lo = (bi // 2) * 2 * UG
            u_hi = ug0 + n_u
            nc.sync.dma_start(
                out=outd[p][:, u_lo * V:u_hi * V],
                in_=stg[0:TBp, u_lo * V:u_hi * V],
            )

    # software-pipelined emission: backs are paced against fronts by
    # modeled engine time (ACT ns for fronts, DVE ns for backs) so the
    # DVE lags ACT by a constant fraction across width changes
    def act_cost(p, fi):
        _t, n_t = fronts[p][fi]
        return n_t * geom[p][4] * 0.8333 + 190.0

    total_act = sum(act_cost(p, fi)
                    for p in range(len(geom)) for fi in range(len(fronts[p])))
    total_dve = sum(back_cost(p, bi)
                    for p in range(len(geom)) for bi in range(n_backs(p)))
    ratio = total_dve / max(total_act, 1.0)

    # backs of the final two rows execute after (or right at) the last
    # tanh: they are tail drain, split across ScalarE+VectorE below, and
    # excluded from the paced in-flight drain
    tail_rows = {len(geom) - 1}
    if len(geom) > 1:
        tail_rows.add(len(geom) - 2)

    pending = []
    act_emitted = 0.0
    dve_emitted = 0.0
    for p in range(len(geom)):
        for fi in range(len(fronts[p])):
            front_mm(p, fi)
            front_tanh(p, fi)
            act_emitted += act_cost(p, fi)
            while pending and dve_emitted < act_emitted * ratio:
                bp, bb = pending.pop(0)
                back(bp, bb, False)
                dve_emitted += back_cost(bp, bb)
        if p not in tail_rows:
            pending.extend((p, bi) for bi in range(n_backs(p)))
    # drain. In the last rep ACT is idle after the final tanh, so
    # alternating evacuations across ScalarE+VectorE halves that tail;
    # in earlier reps the next rep's tanhs already queue on ACT, so
    # scalar copies would lengthen the bottleneck queue — keep them off.
    tail = pending + [(p, bi) for p in sorted(tail_rows)
                      for bi in range(n_backs(p))]
    for i, (bp, bb) in enumerate(tail):
        back(bp, bb, last_rep and i % 2 == 1)


def _host_pred_bias(targets_b, emb, pw1, pb1, pw2, pb2, jw1, jb1):
    """bias[u, j] = (pred @ jw1[E:] + jb1)[u, j] for the U1 joint positions."""
    ext = np.concatenate([np.full(H, BLANK, np.int64), targets_b.astype(np.int64)])
    e = np.concatenate([emb[ext[1:U1 + 1]], emb[ext[0:U1]]], axis=1)  # [101, 512]
    h = np.tanh(e @ pw1 + pb1)
    pred = np.tanh(h @ pw2 + pb2)
    return (pred @ jw1[E:] + jb1).astype(np.float32)  # [101, 128]


def _make_sel(wrow, u0, ws):
    """Selection rhs for a section: K rows = [16 ep rows ; wrow bias rows];
    column (tl, v) sums ep row tl and bias row u0+v."""
    sel = np.zeros((SPAN + wrow, SPAN * ws), np.float16)
    for tl in range(SPAN):
        sel[tl, tl * ws:(tl + 1) * ws] = 1.0
        sel[SPAN + u0:SPAN + u0 + ws, tl * ws:(tl + 1) * ws] += \
            np.eye(ws, dtype=np.float16)
    return sel


def _make_in_maps(encoder_states, targets, emb, pw1, pb1, pw2, pb2, jw1, jb1,
                  jw2, U1e, rows):
    encoder_states = np.asarray(encoder_states, dtype=np.float32)
    jw1 = np.asarray(jw1, dtype=np.float32)
    jw2_np = np.ascontiguousarray(np.asarray(jw2, dtype=np.float32)).astype(np.float16)
    jw1enc = np.ascontiguousarray(jw1[:E])

    # host: encoder projection (fp32 GEMM, zero-padded to span multiple)
    Tpad = T + SPAN
    eproj = np.zeros((B, Tpad, J), np.float16)
    for b in range(B):
        eproj[b, :T] = (encoder_states[b] @ jw1enc).astype(np.float16)
    bias_all = np.empty((B, U1, J), np.float16)
    for b in range(B):
        bias_all[b] = _host_pred_bias(
            np.asarray(targets[b]), np.asarray(emb, np.float32),
            np.asarray(pw1, np.float32), np.asarray(pb1, np.float32),
            np.asarray(pw2, np.float32), np.asarray(pb2, np.float32),
            jw1, np.asarray(jb1, np.float32),
        ).astype(np.float16)

    secs = _sections(rows)
    sel_keys = []
    for (_r, _TB, Wr, u0, Ws, _vf) in secs:
        if (Wr, u0, Ws) not in sel_keys:
            sel_keys.append((Wr, u0, Ws))
    sels = {k: _make_sel(*k) for k in sel_keys}

    in_maps = []
    for c in range(N_CORES):
        m = {"jw2d": jw2_np}
        for k in sel_keys:
            m[f"selw{k[0]}_{k[1]}_{k[2]}"] = sels[k]
        for r, (TBp, Wp, _vf, assign) in enumerate(rows):
            KC = SPAN + Wp
            combd = np.zeros((KC, (TBp // SPAN) * J), np.float16)
            for k, (b, ts) in enumerate(assign[c]):
                if b < 0:
                    continue
                combd[0:SPAN, k * J:(k + 1) * J] = eproj[b, ts:ts + SPAN]
                combd[SPAN:KC, k * J:(k + 1) * J] = bias_all[b, 0:Wp]
            m[f"combd{r}"] = combd
        in_maps.append(m)
    return in_maps


def kernel(encoder_states, encoder_states_size, targets, targets_size,
           emb, pw1, pb1, pw2, pb2, jw1, jb1, jw2, jb2):
    tsz = np.asarray(encoder_states_size).astype(np.int64)
    usz = np.asarray(targets_size).astype(np.int64)
    plan = _plan(tsz, usz)
    if plan is None:  # no valid lattice positions anywhere
        return np.zeros((B, T, U1, V), np.float32)
    U1e, rows = plan
    geom = tuple(_sections(rows))

    if _CACHE.get("geom") != geom:
        _CACHE.clear()
        _CACHE["geom"] = geom
    if "nc" not in _CACHE:
        _CACHE["nc"] = _build_program(reps=1, geom=geom)
    nc = _CACHE["nc"]

    in_maps = _make_in_maps(encoder_states, targets, emb, pw1, pb1, pw2, pb2,
                            jw1, jb1, jw2, U1e, rows)
    _CACHE["in_maps"] = in_maps
    res = run_bass_kernel_spmd(nc, in_maps, core_ids=list(range(N_CORES)))

    jb2f = np.asarray(jb2, np.float32)
    usz1 = usz + 1
    out = np.zeros((B, T, U1, V), np.float32)
    for c in range(N_CORES):
        for p, (r, TBp, _Wr, u0, Ws, vf) in enumerate(geom):
            assign = rows[r][3]
            dev = res.results[c][f"out{p}"]
            if vf:
                dev = dev.reshape(V, TBp, Ws).transpose(1, 2, 0)  # [t, u, v]
            else:
                dev = dev.reshape(TBp, Ws, V)
            for k, (b, ts) in enumerate(assign[c]):
                if b < 0:
                    continue
                nt = min(SPAN, int(tsz[b]) - ts)
                w = min(u0 + Ws, int(usz1[b])) - u0
                if w <= 0:
                    continue
                blk = dev[k * SPAN:k * SPAN + nt, :w, :]
                out[b, ts:ts + nt, u0:u0 + w, :] = \
                    blk.astype(np.float32) + jb2f
    return out


# revision 23
# speedup vs baseline: 1.5021x; 1.0105x over previous
"""FFNN-Transducer joint-lattice kernel for 8 Trainium2 NeuronCores.

Sorted-span-row decomposition: the unit of work is a 16-frame encoder
span (b, ts). All spans of the batch are sorted by target length
(usz+1) descending and packed into rows of 64 spans (8 cores x 8
spans); each row's joint width W is the max usz+1 within the row, so
u-padding adapts to the data with no fixed phase structure. Spans from
different samples share a row: the combined lhsT carries, per span, 16
encoder-projection rows plus that span's own prediction-bias rows, so
blocks mix samples freely and the t-axis is packed at 16-frame
granularity. The final partial row runs with TB = 16*ceil(left/8)
frames; its joint matmul is emitted v-partitioned (out[v, (t,u)]) so
the drain tail stays ~1us.

Per tile the device computes
    out[t,u,:] = tanh(enc_proj[t,:] + pred_bias[u,:]) @ jw2
with the tiny prediction network and the encoder projection enc@jw1[:E]
(~0.5% of FLOPs) on host.

Device pipeline per core, per row:
  PE:   per CH-frame chunk one "selection" matmul materializing
        A[j,(t,u)] = enc_proj[t,j] + bias[u,j] in PSUM; per u one
        [128x128] x [128x88] joint matmul (t-form), or jw2-stationary
        [128x88]^T @ hid for the small tail row (v-form).
  ACT:  batched tanh PSUM->SBUF fp16, one op per 3-bank A tile. A
        warmup tanh at t=0 preloads the activation table during the
        initial DMAs.
  DVE:  PSUM->SBUF evacuation (fp32->fp16); back-emission is paced by
        modeled ACT/DVE time so neither engine starves at row edges.
  DMA:  outputs streamed per u-group pair on the sync queue; selection
        matrices ride the otherwise-idle gpsimd (SWDGE) queue.

TRN2 fp32 matmul runs at 1/4 rate, so all TensorE-facing tensors are
fp16; PSUM stays fp32. jb2-add and the ragged scatter are host
epilogues.
"""

import os
import sys

for _p in ("/opt/trn_rl_repo", "/root/.axon_site/_ro/trn_rl_repo"):
    if os.path.isdir(_p) and _p not in sys.path:
        sys.path.append(_p)

import numpy as np

import concourse.bass as bass
import concourse.tile as tile
from concourse import bacc, mybir
from concourse.bass_utils import run_bass_kernel_spmd

# Problem dims (hardcoded per contract)
B, T, E = 8, 1000, 512
U = 100
U1 = U + 1
H, D, P = 2, 256, 256
J, V = 128, 88
BLANK = V - 1
N_CORES = 8

SPAN = 16           # t-frames per span (lhsT packing unit)
SPB = 8             # spans per core per full row (TB = 128)
UG = 5              # u-steps per M-PSUM bank in t-form backs

F32 = mybir.dt.float32
F16 = mybir.dt.float16

HID_BUFS = 4        # hid tile rotation depth (SBUF)
STG_BUFS = 3        # staging tile rotation depth (SBUF)
SPLIT_MIN_REST = 24  # u-split wide rows only if remainder >= this

_CACHE = {}


def _ch_for(w):
    """Largest divisor of SPAN with ch*w <= 512 (PSUM-bank column limit)."""
    ch = SPAN
    while ch > 1 and ch * w > 512:
        ch //= 2
    return ch


def _fronts_for(w, tb):
    """(t_off, n_t) A-tile steps covering tb frames: 3 chunks of CH each."""
    ft = 3 * _ch_for(w)
    steps, t = [], 0
    while t < tb:
        n = min(ft, tb - t)
        steps.append((t, n))
        t += n
    return steps


def _plan(tsz, usz):
    """Sort spans by target length, pack into rows of 64.

    Returns (U1e, rows); rows = [(TB, W, vform, assign)] where assign
    is the per-core list of (sample, t_start) spans (-1 = dummy).
    """
    usz1 = usz + 1
    spans = [(b, ts) for b in range(B) for ts in range(0, int(tsz[b]), SPAN)]
    if not spans:
        return None
    spans.sort(key=lambda s: (-int(usz1[s[0]]), s[0], s[1]))
    per_row = N_CORES * SPB
    rows = []
    i = 0
    while i < len(spans):
        chunk = spans[i:i + per_row]
        spr = (len(chunk) + N_CORES - 1) // N_CORES
        W = int(usz1[chunk[0][0]])
        chunk = chunk + [(-1, 0)] * (N_CORES * spr - len(chunk))
        assign = [chunk[c * spr:(c + 1) * spr] for c in range(N_CORES)]
        TB = spr * SPAN
        vform = TB < V  # short rows evacuate fewer cols v-partitioned
        rows.append((TB, W, vform, assign))
        i += per_row
    return int(usz1.max()), rows


def _sections(rows):
    """Split wide rows at u=64 so A-chunks fill PSUM banks (CH stays >= 8,
    fewer/larger ACT ops). Each section shares its row's comb lhsT; the
    selection matrix picks the section's bias rows.

    Returns [(row_id, TB, Wrow, u0, Ws, vform)].
    """
    secs = []
    for r, (TB, W, vform, _assign) in enumerate(rows):
        if W > 64 and W - 64 >= SPLIT_MIN_REST and not vform:
            # (64, rest) halves CH-chunk count: 6+3 ACT ops per 128
            # frames instead of 11. Only when the remainder is wide —
            # each extra section boundary costs ~0.3us of ACT idle.
            secs.append((r, TB, W, 0, 64, False))
            secs.append((r, TB, W, 64, W - 64, False))
        else:
            secs.append((r, TB, W, 0, W, vform))
    return secs


def _build_program(reps=1, geom=None):
    if geom is None:
        geom = _CACHE["geom"]
    # geom: tuple of (row_id, TB, Wrow, u0, Ws, vform) per section
    nc = bacc.Bacc("TRN2", target_bir_lowering=False, debug=False)

    jw2d = nc.dram_tensor("jw2d", [J, V], F16, kind="ExternalInput").ap()
    row_ids = []
    row_dims = {}
    sel_keys = []
    outd = []
    for p, (r, TBp, Wr, u0, Ws, vf) in enumerate(geom):
        if r not in row_ids:
            row_ids.append(r)
            row_dims[r] = (TBp, Wr)
        key = (Wr, u0, Ws)
        if key not in sel_keys:
            sel_keys.append(key)
        oshape = [V, TBp * Ws] if vf else [TBp, Ws * V]
        outd.append(nc.dram_tensor(
            f"out{p}", oshape, F16, kind="ExternalOutput").ap())
    combd = {r: nc.dram_tensor(
        f"combd{r}", [SPAN + row_dims[r][1], (row_dims[r][0] // SPAN) * J],
        F16, kind="ExternalInput").ap() for r in row_ids}
    seld = {k: nc.dram_tensor(f"selw{k[0]}_{k[1]}_{k[2]}",
                              [SPAN + k[0], SPAN * k[2]], F16,
                              kind="ExternalInput").ap()
            for k in sel_keys}

    hid_max = max(TBp * Ws for (_r, TBp, _Wr, _u0, Ws, _vf) in geom)
    stg_max = max((Ws * V if not vf else TBp * Ws)
                  for (_r, TBp, _Wr, _u0, Ws, vf) in geom)

    with tile.TileContext(nc) as tc:
        with (
            tc.tile_pool(name="singles", bufs=1) as singles,
            tc.tile_pool(name="hidp", bufs=HID_BUFS) as hidp,
            tc.tile_pool(name="stgp", bufs=STG_BUFS) as stgp,
            tc.tile_pool(name="psA", bufs=2, space="PSUM") as psA,
            tc.tile_pool(name="psM", bufs=2, space="PSUM") as psM,
        ):
            # warmup: preload the tanh table set while the first DMAs fly
            warm = singles.tile([128, 2], F16, tag="warm", name="warm")
            warm2 = singles.tile([128, 2], F16, tag="warm2", name="warm2")
            nc.vector.memset(warm[:, :], 0.0)
            nc.scalar.activation(out=warm2[:, :], in_=warm[:, :],
                                 func=mybir.ActivationFunctionType.Tanh)

            comb_sb = {}
            sel_sb = {}
            for r in row_ids:
                TBr, Wr = row_dims[r]
                comb_sb[r] = singles.tile(
                    [SPAN + Wr, (TBr // SPAN) * J], F16, tag=f"comb{r}",
                    name=f"comb_t{r}")
            for k in sel_keys:
                sel_sb[k] = singles.tile([SPAN + k[0], SPAN * k[2]], F16,
                                         tag=f"selw{k}", name=f"sel_t{k}")
            jw2_sb = singles.tile([J, V], F16, tag="jw2", name="jw2_sb")

            # first row's inputs first (parallel queues: comb on sync,
            # sel on the idle gpsimd/SWDGE queue), then everything else;
            # the first span's lhsT slice leads so the pipeline starts
            # after one tiny DMA instead of the full row load
            r0 = row_ids[0]
            nc.sync.dma_start(out=comb_sb[r0][:, 0:J], in_=combd[r0][:, 0:J])
            nc.sync.dma_start(out=comb_sb[r0][:, J:], in_=combd[r0][:, J:])
            nc.gpsimd.dma_start(out=sel_sb[sel_keys[0]][:, :],
                                in_=seld[sel_keys[0]][:, :])
            nc.sync.dma_start(out=jw2_sb[:, :], in_=jw2d[:, :])
            for k in sel_keys[1:]:
                nc.gpsimd.dma_start(out=sel_sb[k][:, :], in_=seld[k][:, :])
            for r in row_ids[1:]:
                nc.sync.dma_start(out=comb_sb[r][:, :], in_=combd[r][:, :])

            for rep in range(reps):
                _emit_rep(nc, hidp, stgp, psA, psM, comb_sb, jw2_sb, sel_sb,
                          outd, rep, geom, hid_max, stg_max,
                          last_rep=(rep == reps - 1))

    nc.compile()
    return nc


def _emit_rep(nc, hidp, stgp, psA, psM, comb_sb, jw2_sb, sel_sb, outd, rep,
              geom, hid_max, stg_max, last_rep=True):
    fronts = [_fronts_for(Ws, TBp) for (_r, TBp, _Wr, _u0, Ws, _vf) in geom]
    chs = [_ch_for(Ws) for (_r, _TB, _Wr, _u0, Ws, _vf) in geom]

    hid_t, stg_t, A_t = {}, {}, {}

    def front_mm(p, fi):
        r, TBp, Wr, u0, Ws, _vf = geom[p]
        CH = chs[p]
        t_off, n_t = fronts[p][fi]
        if fi == 0:
            hid_t[p] = hidp.tile([128, hid_max], F16, tag="hid",
                                 name=f"hid{rep}_{p}")
        A = psA.tile([128, 1536], F32, tag="A", name=f"A{rep}_{p}_{fi}")
        A_t[(p, fi)] = A
        sel = sel_sb[(Wr, u0, Ws)]
        for c in range(n_t // CH):
            tg = t_off + c * CH
            sp, tl = tg // SPAN, tg % SPAN
            nc.tensor.matmul(
                A[:, c * 512:c * 512 + CH * Ws],
                comb_sb[r][:, sp * J:(sp + 1) * J],
                sel[:, tl * Ws:(tl + CH) * Ws],
                start=True,
                stop=True,
            )

    def front_tanh(p, fi):
        _r, _TBp, _Wr, _u0, Ws, _vf = geom[p]
        CH = chs[p]
        t_off, n_t = fronts[p][fi]
        nch = n_t // CH
        A = A_t.pop((p, fi))
        nc.scalar.activation(
            out=hid_t[p][:, t_off * Ws:(t_off + n_t) * Ws].rearrange(
                "p (c x) -> p c x", c=nch),
            in_=A.rearrange("p (c x) -> p c x", c=3)[:, 0:nch, 0:CH * Ws],
            func=mybir.ActivationFunctionType.Tanh,
        )

    def n_backs(p):
        _r, TBp, _Wr, _u0, Ws, vf = geom[p]
        return ((TBp * Ws + 511) // 512) if vf else ((Ws + UG - 1) // UG)

    def back_cost(p, bi):
        _r, TBp, _Wr, _u0, Ws, vf = geom[p]
        if vf:
            n = min(512, TBp * Ws - bi * 512)
            return n * 1.05 + 130
        n_u = min(UG, Ws - bi * UG)
        return n_u * V * 1.05 + 130

    def back(p, bi, use_scalar):
        _r, TBp, _Wr, _u0, Ws, vf = geom[p]
        hid2 = hid_t[p]
        if bi == 0:
            stg_t[p] = stgp.tile([128, stg_max], F16, tag="stg",
                                 name=f"stg{rep}_{p}")
        stg = stg_t[p]
        M = psM.tile([128, 512], F32, tag="M", name=f"M{rep}_{p}_{bi}")
        cp = nc.scalar.copy if use_scalar else nc.vector.tensor_copy
        if vf:
            c0 = bi * 512
            n = min(512, TBp * Ws - c0)
            nc.tensor.matmul(
                M[0:V, 0:n],
                jw2_sb[:, :],
                hid2[:, c0:c0 + n],
                start=True,
                stop=True,
            )
            cp(out=stg[0:V, c0:c0 + n], in_=M[0:V, 0:n])
            if (bi + 1) * 512 >= TBp * Ws:
                nc.sync.dma_start(out=outd[p][:, :],
                                  in_=stg[0:V, 0:TBp * Ws])
            return
        NUGp = n_backs(p)
        ug0 = bi * UG
        n_u = min(UG, Ws - ug0)
        hid_ut = hid2[:, 0:TBp * Ws].rearrange("p (t u) -> p u t", u=Ws)
        for i in range(n_u):
            nc.tensor.matmul(
                M[0:TBp, i * V:(i + 1) * V],
                hid_ut[:, ug0 + i, :],
                jw2_sb[:, :],
                start=True,
                stop=True,
            )
        cp(out=stg[0:TBp, ug0 * V:(ug0 + n_u) * V], in_=M[0:TBp, 0:n_u * V])
        if bi % 2 == 1 or bi == NUGp - 1:
            u_lo = (bi // 2) * 2 * UG
            u_hi = ug0 + n_u
            nc.sync.dma_start(
                out=outd[p][:, u_lo * V:u_hi * V],
                in_=stg[0:TBp, u_lo * V:u_hi * V],
            )

    # software-pipelined emission: backs are paced against fronts by
    # modeled engine time (ACT ns for fronts, DVE ns for backs) so the
    # DVE lags ACT by a constant fraction across width changes
    def act_cost(p, fi):
        _t, n_t = fronts[p][fi]
        return n_t * geom[p][4] * 0.8333 + 190.0

    total_act = sum(act_cost(p, fi)
                    for p in range(len(geom)) for fi in range(len(fronts[p])))
    total_dve = sum(back_cost(p, bi)
                    for p in range(len(geom)) for bi in range(n_backs(p)))
    ratio = total_dve / max(total_act, 1.0)

    # backs of the final two rows execute after (or right at) the last
    # tanh: they are tail drain, split across ScalarE+VectorE below, and
    # excluded from the paced in-flight drain
    tail_rows = {len(geom) - 1}
    if len(geom) > 1:
        tail_rows.add(len(geom) - 2)

    pending = []
    act_emitted = 0.0
    dve_emitted = 0.0
    for p in range(len(geom)):
        for fi in range(len(fronts[p])):
            front_mm(p, fi)
            front_tanh(p, fi)
            act_emitted += act_cost(p, fi)
            while pending and dve_emitted < act_emitted * ratio:
                bp, bb = pending.pop(0)
                back(bp, bb, False)
                dve_emitted += back_cost(bp, bb)
        if p not in tail_rows:
            pending.extend((p, bi) for bi in range(n_backs(p)))
    # drain. In the last rep ACT is idle after the final tanh, so
    # alternating evacuations across ScalarE+VectorE halves that tail;
    # in earlier reps the next rep's tanhs already queue on ACT, so
    # scalar copies would lengthen the bottleneck queue — keep them off.
    tail = pending + [(p, bi) for p in sorted(tail_rows)
                      for bi in range(n_backs(p))]
    for i, (bp, bb) in enumerate(tail):
        back(bp, bb, last_rep and i % 2 == 1)


def _host_pred_bias(targets_b, emb, pw1, pb1, pw2, pb2, jw1, jb1):
    """bias[u, j] = (pred @ jw1[E:] + jb1)[u, j] for the U1 joint positions."""
    ext = np.concatenate([np.full(H, BLANK, np.int64), targets_b.astype(np.int64)])
    e = np.concatenate([emb[ext[1:U1 + 1]], emb[ext[0:U1]]], axis=1)  # [101, 512]
    h = np.tanh(e @ pw1 + pb1)
    pred = np.tanh(h @ pw2 + pb2)
    return (pred @ jw1[E:] + jb1).astype(np.float32)  # [101, 128]


def _make_sel(wrow, u0, ws):
    """Selection rhs for a section: K rows = [16 ep rows ; wrow bias rows];
    column (tl, v) sums ep row tl and bias row u0+v."""
    sel = np.zeros((SPAN + wrow, SPAN * ws), np.float16)
    for tl in range(SPAN):
        sel[tl, tl * ws:(tl + 1) * ws] = 1.0
        sel[SPAN + u0:SPAN + u0 + ws, tl * ws:(tl + 1) * ws] += \
            np.eye(ws, dtype=np.float16)
    return sel


def _make_in_maps(encoder_states, targets, emb, pw1, pb1, pw2, pb2, jw1, jb1,
                  jw2, U1e, rows):
    encoder_states = np.asarray(encoder_states, dtype=np.float32)
    jw1 = np.asarray(jw1, dtype=np.float32)
    jw2_np = np.ascontiguousarray(np.asarray(jw2, dtype=np.float32)).astype(np.float16)
    jw1enc = np.ascontiguousarray(jw1[:E])

    # host: encoder projection (fp32 GEMM, zero-padded to span multiple)
    Tpad = T + SPAN
    eproj = np.zeros((B, Tpad, J), np.float16)
    for b in range(B):
        eproj[b, :T] = (encoder_states[b] @ jw1enc).astype(np.float16)
    bias_all = np.empty((B, U1, J), np.float16)
    for b in range(B):
        bias_all[b] = _host_pred_bias(
            np.asarray(targets[b]), np.asarray(emb, np.float32),
            np.asarray(pw1, np.float32), np.asarray(pb1, np.float32),
            np.asarray(pw2, np.float32), np.asarray(pb2, np.float32),
            jw1, np.asarray(jb1, np.float32),
        ).astype(np.float16)

    secs = _sections(rows)
    sel_keys = []
    for (_r, _TB, Wr, u0, Ws, _vf) in secs:
        if (Wr, u0, Ws) not in sel_keys:
            sel_keys.append((Wr, u0, Ws))
    sels = {k: _make_sel(*k) for k in sel_keys}

    in_maps = []
    for c in range(N_CORES):
        m = {"jw2d": jw2_np}
        for k in sel_keys:
            m[f"selw{k[0]}_{k[1]}_{k[2]}"] = sels[k]
        for r, (TBp, Wp, _vf, assign) in enumerate(rows):
            KC = SPAN + Wp
            combd = np.zeros((KC, (TBp // SPAN) * J), np.float16)
            for k, (b, ts) in enumerate(assign[c]):
                if b < 0:
                    continue
                combd[0:SPAN, k * J:(k + 1) * J] = eproj[b, ts:ts + SPAN]
                combd[SPAN:KC, k * J:(k + 1) * J] = bias_all[b, 0:Wp]
            m[f"combd{r}"] = combd
        in_maps.append(m)
    return in_maps


def kernel(encoder_states, encoder_states_size, targets, targets_size,
           emb, pw1, pb1, pw2, pb2, jw1, jb1, jw2, jb2):
    tsz = np.asarray(encoder_states_size).astype(np.int64)
    usz = np.asarray(targets_size).astype(np.int64)
    plan = _plan(tsz, usz)
    if plan is None:  # no valid lattice positions anywhere
        return np.zeros((B, T, U1, V), np.float32)
    U1e, rows = plan
    geom = tuple(_sections(rows))

    if _CACHE.get("geom") != geom:
        _CACHE.clear()
        _CACHE["geom"] = geom
    if "nc" not in _CACHE:
        _CACHE["nc"] = _build_program(reps=1, geom=geom)
    nc = _CACHE["nc"]

    in_maps = _make_in_maps(encoder_states, targets, emb, pw1, pb1, pw2, pb2,
                            jw1, jb1, jw2, U1e, rows)
    _CACHE["in_maps"] = in_maps
    res = run_bass_kernel_spmd(nc, in_maps, core_ids=list(range(N_CORES)))

    jb2f = np.asarray(jb2, np.float32)
    usz1 = usz + 1
    out = np.zeros((B, T, U1, V), np.float32)
    for c in range(N_CORES):
        for p, (r, TBp, _Wr, u0, Ws, vf) in enumerate(geom):
            assign = rows[r][3]
            dev = res.results[c][f"out{p}"]
            if vf:
                dev = dev.reshape(V, TBp, Ws).transpose(1, 2, 0)  # [t, u, v]
            else:
                dev = dev.reshape(TBp, Ws, V)
            for k, (b, ts) in enumerate(assign[c]):
                if b < 0:
                    continue
                nt = min(SPAN, int(tsz[b]) - ts)
                w = min(u0 + Ws, int(usz1[b])) - u0
                if w <= 0:
                    continue
                blk = dev[k * SPAN:k * SPAN + nt, :w, :]
                out[b, ts:ts + nt, u0:u0 + w, :] = \
                    blk.astype(np.float32) + jb2f
    return out


# revision 25
# speedup vs baseline: 2.9252x; 1.9474x over previous
"""FFNN-Transducer joint-lattice kernel for 8 Trainium2 NeuronCores.

Sorted-span-row decomposition: the unit of work is a 16-frame encoder
span (b, ts). All spans of the batch are sorted by target length
(usz+1) descending and packed into rows of 64 spans (8 cores x 8
spans); each row's joint width W is the max usz+1 within the row, so
u-padding adapts to the data with no fixed phase structure. Spans from
different samples share a row: the combined lhsT carries, per span, 16
encoder-projection rows plus that span's own prediction-bias rows, so
blocks mix samples freely and the t-axis is packed at 16-frame
granularity. The final partial row runs with TB = 16*ceil(left/8)
frames; its joint matmul is emitted v-partitioned (out[v, (t,u)]) so
the drain tail stays ~1us.

Per tile the device computes
    out[t,u,:] = tanh(enc_proj[t,:] + pred_bias[u,:]) @ jw2
with the tiny prediction network and the encoder projection enc@jw1[:E]
(~0.5% of FLOPs) on host.

Device pipeline per core, per row:
  PE:   per CH-frame chunk one "selection" matmul materializing
        A[j,(t,u)] = enc_proj[t,j] + bias[u,j] in PSUM; per u one
        [128x128] x [128x88] joint matmul (t-form), or jw2-stationary
        [128x88]^T @ hid for the small tail row (v-form).
  ACT:  batched tanh PSUM->SBUF fp16, one op per 3-bank A tile. A
        warmup tanh at t=0 preloads the activation table during the
        initial DMAs.
  DVE:  PSUM->SBUF evacuation (fp32->fp16); back-emission is paced by
        modeled ACT/DVE time so neither engine starves at row edges.
  DMA:  outputs streamed per u-group pair on the sync queue; selection
        matrices ride the otherwise-idle gpsimd (SWDGE) queue.

TRN2 fp32 matmul runs at 1/4 rate, so all TensorE-facing tensors are
fp16; PSUM stays fp32. jb2-add and the ragged scatter are host
epilogues.
"""

import os
import sys

for _p in ("/opt/trn_rl_repo", "/root/.axon_site/_ro/trn_rl_repo"):
    if os.path.isdir(_p) and _p not in sys.path:
        sys.path.append(_p)

import numpy as np

import concourse.bass as bass
import concourse.tile as tile
from concourse import bacc, mybir
from concourse.bass_utils import run_bass_kernel_spmd

# Problem dims (hardcoded per contract)
B, T, E = 8, 1000, 512
U = 100
U1 = U + 1
H, D, P = 2, 256, 256
J, V = 128, 88
BLANK = V - 1
N_CORES = 8

SPAN = 16           # t-frames per span (lhsT packing unit)
SPB = 8             # spans per core per full row (TB = 128)
UG = 5              # u-steps per M-PSUM bank in t-form backs

F32 = mybir.dt.float32
F16 = mybir.dt.float16

HID_BUFS = 4        # hid tile rotation depth (SBUF)
STG_BUFS = 3        # staging tile rotation depth (SBUF)
SPLIT_MIN_REST = 24  # u-split wide rows only if remainder >= this

_CACHE = {}


def _ch_for(w):
    """Largest divisor of SPAN with ch*w <= 512 (PSUM-bank column limit)."""
    ch = SPAN
    while ch > 1 and ch * w > 512:
        ch //= 2
    return ch


def _fronts_for(w, tb):
    """(t_off, n_t) A-tile steps covering tb frames, <=3 chunks of CH
    each, sized as evenly as CH granularity allows: a tiny final tile
    lets ACT overtake PE at section boundaries and exposes the PE->ACT
    semaphore latency."""
    ch = _ch_for(w)
    ft = 3 * ch
    nf = (tb + ft - 1) // ft
    nch_total = tb // ch
    steps, t = [], 0
    for i in range(nf):
        nch = (nch_total * (i + 1)) // nf - (nch_total * i) // nf
        n = nch * ch
        steps.append((t, n))
        t += n
    assert t == tb
    return steps


def _plan(tsz, usz):
    """Sort spans by target length, pack into rows of 64.

    Returns (U1e, rows); rows = [(TB, W, vform, assign)] where assign
    is the per-core list of (sample, t_start) spans (-1 = dummy).
    """
    usz1 = usz + 1
    spans = [(b, ts) for b in range(B) for ts in range(0, int(tsz[b]), SPAN)]
    if not spans:
        return None
    spans.sort(key=lambda s: (-int(usz1[s[0]]), s[0], s[1]))
    per_row = N_CORES * SPB
    rows = []
    i = 0
    while i < len(spans):
        chunk = spans[i:i + per_row]
        spr = (len(chunk) + N_CORES - 1) // N_CORES
        W = int(usz1[chunk[0][0]])
        chunk = chunk + [(-1, 0)] * (N_CORES * spr - len(chunk))
        assign = [chunk[c * spr:(c + 1) * spr] for c in range(N_CORES)]
        TB = spr * SPAN
        vform = TB < V  # short rows evacuate fewer cols v-partitioned
        rows.append((TB, W, vform, assign))
        i += per_row
    return int(usz1.max()), rows


def _sections(rows):
    """Split wide rows at u=64 so A-chunks fill PSUM banks (CH stays >= 8,
    fewer/larger ACT ops). Each section shares its row's comb lhsT; the
    selection matrix picks the section's bias rows.

    Returns [(row_id, TB, Wrow, u0, Ws, vform)].
    """
    secs = []
    for r, (TB, W, vform, _assign) in enumerate(rows):
        if W > 64 and W - 64 >= SPLIT_MIN_REST and not vform:
            # (64, rest) halves CH-chunk count: 6+3 ACT ops per 128
            # frames instead of 11. Only when the remainder is wide —
            # each extra section boundary costs ~0.3us of ACT idle.
            secs.append((r, TB, W, 0, 64, False))
            secs.append((r, TB, W, 64, W - 64, False))
        else:
            secs.append((r, TB, W, 0, W, vform))
    return secs


def _build_program(reps=1, geom=None):
    if geom is None:
        geom = _CACHE["geom"]
    # geom: tuple of (row_id, TB, Wrow, u0, Ws, vform) per section
    nc = bacc.Bacc("TRN2", target_bir_lowering=False, debug=False)

    jw2d = nc.dram_tensor("jw2d", [J, V], F16, kind="ExternalInput").ap()
    row_ids = []
    row_dims = {}
    sel_keys = []
    outd = []
    for p, (r, TBp, Wr, u0, Ws, vf) in enumerate(geom):
        if r not in row_ids:
            row_ids.append(r)
            row_dims[r] = (TBp, Wr)
        key = (Wr, u0, Ws)
        if key not in sel_keys:
            sel_keys.append(key)
        oshape = [V, TBp * Ws] if vf else [TBp, Ws * V]
        outd.append(nc.dram_tensor(
            f"out{p}", oshape, F16, kind="ExternalOutput").ap())
    combd = {r: nc.dram_tensor(
        f"combd{r}", [SPAN + row_dims[r][1], (row_dims[r][0] // SPAN) * J],
        F16, kind="ExternalInput").ap() for r in row_ids}
    seld = {k: nc.dram_tensor(f"selw{k[0]}_{k[1]}_{k[2]}",
                              [SPAN + k[0], SPAN * k[2]], F16,
                              kind="ExternalInput").ap()
            for k in sel_keys}

    hid_max = max(TBp * Ws for (_r, TBp, _Wr, _u0, Ws, _vf) in geom)
    stg_max = max((Ws * V if not vf else TBp * Ws)
                  for (_r, TBp, _Wr, _u0, Ws, vf) in geom)

    with tile.TileContext(nc) as tc:
        with (
            tc.tile_pool(name="singles", bufs=1) as singles,
            tc.tile_pool(name="hidp", bufs=HID_BUFS) as hidp,
            tc.tile_pool(name="stgp", bufs=STG_BUFS) as stgp,
            tc.tile_pool(name="psA", bufs=2, space="PSUM") as psA,
            tc.tile_pool(name="psM", bufs=2, space="PSUM") as psM,
        ):
            # warmup: preload the tanh table set while the first DMAs fly
            warm = singles.tile([128, 2], F16, tag="warm", name="warm")
            warm2 = singles.tile([128, 2], F16, tag="warm2", name="warm2")
            nc.vector.memset(warm[:, :], 0.0)
            nc.scalar.activation(out=warm2[:, :], in_=warm[:, :],
                                 func=mybir.ActivationFunctionType.Tanh)

            comb_sb = {}
            sel_sb = {}
            for r in row_ids:
                TBr, Wr = row_dims[r]
                comb_sb[r] = singles.tile(
                    [SPAN + Wr, (TBr // SPAN) * J], F16, tag=f"comb{r}",
                    name=f"comb_t{r}")
            for k in sel_keys:
                sel_sb[k] = singles.tile([SPAN + k[0], SPAN * k[2]], F16,
                                         tag=f"selw{k}", name=f"sel_t{k}")
            jw2_sb = singles.tile([J, V], F16, tag="jw2", name="jw2_sb")

            # first row's inputs first (parallel queues: comb on sync,
            # sel on the idle gpsimd/SWDGE queue), then everything else;
            # the first span's lhsT slice leads so the pipeline starts
            # after one tiny DMA instead of the full row load
            r0 = row_ids[0]
            nc.sync.dma_start(out=comb_sb[r0][:, 0:J], in_=combd[r0][:, 0:J])
            nc.sync.dma_start(out=comb_sb[r0][:, J:], in_=combd[r0][:, J:])
            nc.gpsimd.dma_start(out=sel_sb[sel_keys[0]][:, :],
                                in_=seld[sel_keys[0]][:, :])
            nc.sync.dma_start(out=jw2_sb[:, :], in_=jw2d[:, :])
            for k in sel_keys[1:]:
                nc.gpsimd.dma_start(out=sel_sb[k][:, :], in_=seld[k][:, :])
            for r in row_ids[1:]:
                nc.sync.dma_start(out=comb_sb[r][:, :], in_=combd[r][:, :])

            for rep in range(reps):
                _emit_rep(nc, hidp, stgp, psA, psM, comb_sb, jw2_sb, sel_sb,
                          outd, rep, geom, hid_max, stg_max,
                          last_rep=(rep == reps - 1))

    nc.compile()
    return nc


def _emit_rep(nc, hidp, stgp, psA, psM, comb_sb, jw2_sb, sel_sb, outd, rep,
              geom, hid_max, stg_max, last_rep=True):
    fronts = [_fronts_for(Ws, TBp) for (_r, TBp, _Wr, _u0, Ws, _vf) in geom]
    chs = [_ch_for(Ws) for (_r, _TB, _Wr, _u0, Ws, _vf) in geom]

    hid_t, stg_t, A_t = {}, {}, {}

    def front_mm(p, fi):
        r, TBp, Wr, u0, Ws, _vf = geom[p]
        CH = chs[p]
        t_off, n_t = fronts[p][fi]
        if fi == 0:
            hid_t[p] = hidp.tile([128, hid_max], F16, tag="hid",
                                 name=f"hid{rep}_{p}")
        A = psA.tile([128, 1536], F32, tag="A", name=f"A{rep}_{p}_{fi}")
        A_t[(p, fi)] = A
        sel = sel_sb[(Wr, u0, Ws)]
        for c in range(n_t // CH):
            tg = t_off + c * CH
            sp, tl = tg // SPAN, tg % SPAN
            nc.tensor.matmul(
                A[:, c * 512:c * 512 + CH * Ws],
                comb_sb[r][:, sp * J:(sp + 1) * J],
                sel[:, tl * Ws:(tl + CH) * Ws],
                start=True,
                stop=True,
            )

    def front_tanh(p, fi):
        _r, _TBp, _Wr, _u0, Ws, _vf = geom[p]
        CH = chs[p]
        t_off, n_t = fronts[p][fi]
        nch = n_t // CH
        A = A_t.pop((p, fi))
        nc.scalar.activation(
            out=hid_t[p][:, t_off * Ws:(t_off + n_t) * Ws].rearrange(
                "p (c x) -> p c x", c=nch),
            in_=A.rearrange("p (c x) -> p c x", c=3)[:, 0:nch, 0:CH * Ws],
            func=mybir.ActivationFunctionType.Tanh,
        )

    def n_backs(p):
        _r, TBp, _Wr, _u0, Ws, vf = geom[p]
        return ((TBp * Ws + 511) // 512) if vf else ((Ws + UG - 1) // UG)

    def back_cost(p, bi):
        _r, TBp, _Wr, _u0, Ws, vf = geom[p]
        if vf:
            n = min(512, TBp * Ws - bi * 512)
            return n * 1.05 + 130
        n_u = min(UG, Ws - bi * UG)
        return n_u * V * 1.05 + 130

    def back(p, bi, use_scalar):
        _r, TBp, _Wr, _u0, Ws, vf = geom[p]
        hid2 = hid_t[p]
        if bi == 0:
            stg_t[p] = stgp.tile([128, stg_max], F16, tag="stg",
                                 name=f"stg{rep}_{p}")
        stg = stg_t[p]
        M = psM.tile([128, 512], F32, tag="M", name=f"M{rep}_{p}_{bi}")
        cp = nc.scalar.copy if use_scalar else nc.vector.tensor_copy
        if vf:
            c0 = bi * 512
            n = min(512, TBp * Ws - c0)
            nc.tensor.matmul(
                M[0:V, 0:n],
                jw2_sb[:, :],
                hid2[:, c0:c0 + n],
                start=True,
                stop=True,
            )
            cp(out=stg[0:V, c0:c0 + n], in_=M[0:V, 0:n])
            if (bi + 1) * 512 >= TBp * Ws:
                nc.sync.dma_start(out=outd[p][:, :],
                                  in_=stg[0:V, 0:TBp * Ws])
            return
        NUGp = n_backs(p)
        ug0 = bi * UG
        n_u = min(UG, Ws - ug0)
        hid_ut = hid2[:, 0:TBp * Ws].rearrange("p (t u) -> p u t", u=Ws)
        for i in range(n_u):
            nc.tensor.matmul(
                M[0:TBp, i * V:(i + 1) * V],
                hid_ut[:, ug0 + i, :],
                jw2_sb[:, :],
                start=True,
                stop=True,
            )
        cp(out=stg[0:TBp, ug0 * V:(ug0 + n_u) * V], in_=M[0:TBp, 0:n_u * V])
        if bi % 2 == 1 or bi == NUGp - 1:
            u_lo = (bi // 2) * 2 * UG
            u_hi = ug0 + n_u
            nc.sync.dma_start(
                out=outd[p][:, u_lo * V:u_hi * V],
                in_=stg[0:TBp, u_lo * V:u_hi * V],
            )

    # software-pipelined emission: backs are paced against fronts by
    # modeled engine time (ACT ns for fronts, DVE ns for backs) so the
    # DVE lags ACT by a constant fraction across width changes
    def act_cost(p, fi):
        _t, n_t = fronts[p][fi]
        return n_t * geom[p][4] * 0.8333 + 190.0

    total_act = sum(act_cost(p, fi)
                    for p in range(len(geom)) for fi in range(len(fronts[p])))
    total_dve = sum(back_cost(p, bi)
                    for p in range(len(geom)) for bi in range(n_backs(p)))
    ratio = total_dve / max(total_act, 1.0)

    # backs of the final two rows execute after (or right at) the last
    # tanh: they are tail drain, split across ScalarE+VectorE below, and
    # excluded from the paced in-flight drain
    tail_rows = {len(geom) - 1}
    if len(geom) > 1:
        tail_rows.add(len(geom) - 2)

    pending = []
    act_emitted = 0.0
    dve_emitted = 0.0
    for p in range(len(geom)):
        for fi in range(len(fronts[p])):
            front_mm(p, fi)
            front_tanh(p, fi)
            act_emitted += act_cost(p, fi)
            # cap the drain at 3 backs per front: a section-boundary
            # burst of PE back-matmuls between consecutive A-tile
            # builds would exceed the ACT tile window and starve it
            drained = 0
            while (pending and drained < 3
                   and dve_emitted < act_emitted * ratio):
                bp, bb = pending.pop(0)
                back(bp, bb, False)
                dve_emitted += back_cost(bp, bb)
                drained += 1
        if p not in tail_rows:
            pending.extend((p, bi) for bi in range(n_backs(p)))
    # drain. In the last rep ACT is idle after the final tanh, so
    # alternating evacuations across ScalarE+VectorE halves that tail;
    # in earlier reps the next rep's tanhs already queue on ACT, so
    # scalar copies would lengthen the bottleneck queue — keep them off.
    tail = pending + [(p, bi) for p in sorted(tail_rows)
                      for bi in range(n_backs(p))]
    for i, (bp, bb) in enumerate(tail):
        back(bp, bb, last_rep and i % 2 == 1)


def _host_pred_bias(targets_b, emb, pw1, pb1, pw2, pb2, jw1, jb1):
    """bias[u, j] = (pred @ jw1[E:] + jb1)[u, j] for the U1 joint positions."""
    ext = np.concatenate([np.full(H, BLANK, np.int64), targets_b.astype(np.int64)])
    e = np.concatenate([emb[ext[1:U1 + 1]], emb[ext[0:U1]]], axis=1)  # [101, 512]
    h = np.tanh(e @ pw1 + pb1)
    pred = np.tanh(h @ pw2 + pb2)
    return (pred @ jw1[E:] + jb1).astype(np.float32)  # [101, 128]


def _make_sel(wrow, u0, ws):
    """Selection rhs for a section: K rows = [16 ep rows ; wrow bias rows];
    column (tl, v) sums ep row tl and bias row u0+v."""
    sel = np.zeros((SPAN + wrow, SPAN * ws), np.float16)
    for tl in range(SPAN):
        sel[tl, tl * ws:(tl + 1) * ws] = 1.0
        sel[SPAN + u0:SPAN + u0 + ws, tl * ws:(tl + 1) * ws] += \
            np.eye(ws, dtype=np.float16)
    return sel


def _make_in_maps(encoder_states, targets, emb, pw1, pb1, pw2, pb2, jw1, jb1,
                  jw2, U1e, rows):
    encoder_states = np.asarray(encoder_states, dtype=np.float32)
    jw1 = np.asarray(jw1, dtype=np.float32)
    jw2_np = np.ascontiguousarray(np.asarray(jw2, dtype=np.float32)).astype(np.float16)
    jw1enc = np.ascontiguousarray(jw1[:E])

    # host: encoder projection (fp32 GEMM, zero-padded to span multiple)
    Tpad = T + SPAN
    eproj = np.zeros((B, Tpad, J), np.float16)
    for b in range(B):
        eproj[b, :T] = (encoder_states[b] @ jw1enc).astype(np.float16)
    bias_all = np.empty((B, U1, J), np.float16)
    for b in range(B):
        bias_all[b] = _host_pred_bias(
            np.asarray(targets[b]), np.asarray(emb, np.float32),
            np.asarray(pw1, np.float32), np.asarray(pb1, np.float32),
            np.asarray(pw2, np.float32), np.asarray(pb2, np.float32),
            jw1, np.asarray(jb1, np.float32),
        ).astype(np.float16)

    secs = _sections(rows)
    sel_keys = []
    for (_r, _TB, Wr, u0, Ws, _vf) in secs:
        if (Wr, u0, Ws) not in sel_keys:
            sel_keys.append((Wr, u0, Ws))
    sels = {k: _make_sel(*k) for k in sel_keys}

    in_maps = []
    for c in range(N_CORES):
        m = {"jw2d": jw2_np}
        for k in sel_keys:
            m[f"selw{k[0]}_{k[1]}_{k[2]}"] = sels[k]
        for r, (TBp, Wp, _vf, assign) in enumerate(rows):
            KC = SPAN + Wp
            combd = np.zeros((KC, (TBp // SPAN) * J), np.float16)
            for k, (b, ts) in enumerate(assign[c]):
                if b < 0:
                    continue
                combd[0:SPAN, k * J:(k + 1) * J] = eproj[b, ts:ts + SPAN]
                combd[SPAN:KC, k * J:(k + 1) * J] = bias_all[b, 0:Wp]
            m[f"combd{r}"] = combd
        in_maps.append(m)
    return in_maps


def kernel(encoder_states, encoder_states_size, targets, targets_size,
           emb, pw1, pb1, pw2, pb2, jw1, jb1, jw2, jb2):
    tsz = np.asarray(encoder_states_size).astype(np.int64)
    usz = np.asarray(targets_size).astype(np.int64)
    plan = _plan(tsz, usz)
    if plan is None:  # no valid lattice positions anywhere
        return np.zeros((B, T, U1, V), np.float32)
    U1e, rows = plan
    geom = tuple(_sections(rows))

    if _CACHE.get("geom") != geom:
        _CACHE.clear()
        _CACHE["geom"] = geom
    if "nc" not in _CACHE:
        _CACHE["nc"] = _build_program(reps=1, geom=geom)
    nc = _CACHE["nc"]

    in_maps = _make_in_maps(encoder_states, targets, emb, pw1, pb1, pw2, pb2,
                            jw1, jb1, jw2, U1e, rows)
    _CACHE["in_maps"] = in_maps
    res = run_bass_kernel_spmd(nc, in_maps, core_ids=list(range(N_CORES)))

    jb2f = np.asarray(jb2, np.float32)
    usz1 = usz + 1
    out = np.zeros((B, T, U1, V), np.float32)
    for c in range(N_CORES):
        for p, (r, TBp, _Wr, u0, Ws, vf) in enumerate(geom):
            assign = rows[r][3]
            dev = res.results[c][f"out{p}"]
            if vf:
                dev = dev.reshape(V, TBp, Ws).transpose(1, 2, 0)  # [t, u, v]
            else:
                dev = dev.reshape(TBp, Ws, V)
            for k, (b, ts) in enumerate(assign[c]):
                if b < 0:
                    continue
                nt = min(SPAN, int(tsz[b]) - ts)
                w = min(u0 + Ws, int(usz1[b])) - u0
                if w <= 0:
                    continue
                blk = dev[k * SPAN:k * SPAN + nt, :w, :]
                out[b, ts:ts + nt, u0:u0 + w, :] = \
                    blk.astype(np.float32) + jb2f
    return out
